# revision 6
# baseline (speedup 1.0000x reference)
"""Trainium2 Bass kernel for a binarized transformer block (BiT-style).

Block (per batch element, forward only):
    h   = LN1(x);  s1 = sign(h)
    z   = s1 @ sign(w_qkv)^T          (alpha>0 dropped: only signs consumed)
    q,k,v = sign(z) split into heads  (+-1)
    S   = q @ k^T  (integer);  T = (S>0)   <- forward value of softmax-STE
    O   = T @ v    (integer);  so = sign(O)
    x1  = x + ls1*(so @ (a_p*sign(w_proj))^T + b_proj)
    h2  = LN2(x1)
    m   = gelu(h2 @ sign(w_fc1)^T * a1 + b1)
    out = x1 + ls2*(m @ (a2*sign(w_fc2))^T + b_fc2)

All binary matmuls are exact: +-1/{0,2} operands in fp8, fp32 PSUM
accumulation of integers.  Thresholds are Sign(2z+1) on odd integers, so
never evaluated at 0.  Sharding: batch 8 -> one element per NeuronCore,
no collectives.

v2 structure:
  A: LN1 -> DMA-xbar transposes (no PE transposes) -> qkv (q/k o-major into
     merged kzT/qkT; v token-major into zero-padded vza/vzb) -> colsums.
  X: attention over two 512-col n-halves; S via tile_position row-split
     (K=64 heads run concurrently in array halves); S-binarize evacs
     split ACT/DVE per (pair, half); O interleaved one pair behind.
  Y: proj+LN2 per token tile (h2 via DMA-xbar transpose), fc1 in n-halves
     overlapping the LN2 chains of later tiles, fc2 + residual + store.
"""

import sys
import os

sys.path.insert(0, "/opt/trn_rl_repo")

import numpy as np
import ml_dtypes
from contextlib import ExitStack
from dataclasses import dataclass

from concourse import bass, bacc, mybir, tile

P = 128
C = 768
CT = C // P          # 6 channel chunks
H = 12
HD = 64
HID = 3072
HT = HID // P        # 24 hidden chunks
OC = 3 * C           # 2304
B = 8
N_CORES = 8
NT = 8
N = NT * P           # 1024
NH = N // 2          # 512 (attention n-half)

F32 = mybir.dt.float32
BF16 = mybir.dt.bfloat16
FP8 = mybir.dt.float8e4
AF = mybir.ActivationFunctionType
AL = mybir.AluOpType
DR = mybir.MatmulPerfMode.DoubleRow

# pairs whose S-binarize runs on ScalarE (+-1 encoding, colsum-corrected
# O bias); the rest run on VectorE ({0,2} encoding, bias=1).  Granularity
# is per (pair); tune for ACT/DVE balance.
ACT_PAIRS = frozenset({1, 4})

# dev hook: CoreSim has no Gelu; dev_sim swaps this for Tanh on both sides.
GELU_FN = AF.Gelu


@dataclass(frozen=True)
class Cfg:
    nt: int = 8            # token tiles of 128 per core
    ln1_fast: bool = True  # ln1_b == 0 and ln1_g > 0 elementwise
    ln2_fast: bool = True  # ln2_g == 1 and ln2_b == 0
    has_cp2: bool = False  # ls1*b_proj != 0
    has_c2: bool = False   # ls2*b_fc2 != 0


def _nchunks(n, step=512):
    out = []
    i = 0
    while i < n:
        out.append((i, min(step, n - i)))
        i += step
    return out


def build_program_v2(cfg: Cfg, dbg=False):
    """Restructured per-core Bass program (fast cfg only)."""
    assert cfg.nt == NT and cfg.ln1_fast and cfg.ln2_fast
    nt = NT

    dbg_t = {}

    nc = bacc.Bacc("TRN2", target_bir_lowering=False, debug=False,
                   enable_asserts=False, num_devices=N_CORES)

    def dbg_dump(name, ap):
        if not dbg:
            return
        d = nc.dram_tensor(f"dbg_{name}", list(ap.shape), ap.dtype,
                           kind="ExternalOutput").ap()
        dbg_t[name] = d
        nc.sync.dma_start(d, ap)

    # ---- DRAM I/O -------------------------------------------------------
    x_d = nc.dram_tensor("x", [N, C], F32, kind="ExternalInput").ap()
    wqkvT_d = nc.dram_tensor("wqkvT", [C, OC], FP8, kind="ExternalInput").ap()
    wpT_d = nc.dram_tensor("wpT", [C, C], FP8, kind="ExternalInput").ap()
    w1T_d = nc.dram_tensor("w1T", [C, HID], FP8, kind="ExternalInput").ap()
    w2T_d = nc.dram_tensor("w2T", [HID, C], FP8, kind="ExternalInput").ap()
    a1s_d = nc.dram_tensor("a1s", [P, HT], F32, kind="ExternalInput").ap()
    b1s_d = nc.dram_tensor("b1s", [P, HT], F32, kind="ExternalInput").ap()
    cp1_d = nc.dram_tensor("cp1r", [P, C], F32, kind="ExternalInput").ap()
    c1_d = nc.dram_tensor("c1r", [P, C], F32, kind="ExternalInput").ap()
    opt_d = {}
    if cfg.has_cp2:
        opt_d["cp2r"] = nc.dram_tensor("cp2r", [P, C], F32, kind="ExternalInput").ap()
    if cfg.has_c2:
        opt_d["c2r"] = nc.dram_tensor("c2r", [P, C], F32, kind="ExternalInput").ap()
    out_d = nc.dram_tensor("out", [N, C], F32, kind="ExternalOutput").ap()

    with tile.TileContext(nc) as tc, ExitStack() as ctx:
        pc = ctx.enter_context(tc.tile_pool(name="const", bufs=1))
        px = ctx.enter_context(tc.tile_pool(name="xp", bufs=1))
        pwbig = ctx.enter_context(tc.tile_pool(name="wbig", bufs=2))
        pwp = ctx.enter_context(tc.tile_pool(name="wp", bufs=1))
        ptok = ctx.enter_context(tc.tile_pool(name="tok", bufs=2))   # s1bf/h2bf
        ptb = ctx.enter_context(tc.tile_pool(name="tb", bufs=3))     # xbar staging
        pch = ctx.enter_context(tc.tile_pool(name="ch", bufs=2))     # s1T/h2T fp8
        pqk = ctx.enter_context(tc.tile_pool(name="qk", bufs=1))
        pv = ctx.enter_context(tc.tile_pool(name="vp", bufs=1))
        pst = ctx.enter_context(tc.tile_pool(name="st", bufs=2))     # S fp8 tiles
        pmg = ctx.enter_context(tc.tile_pool(name="mg", bufs=1))     # gelu out
        pstat = ctx.enter_context(tc.tile_pool(name="stat", bufs=1))

        # PSUM: p2 = 2 slots x 2 banks (qk/v psums, S pairs, fc1);
        #       p1 = 4 slots x 1 bank (O, colsum, proj/fc2 halves, warm)
        p2 = ctx.enter_context(
            tc.tile_pool(name="p2", bufs=2, space=bass.MemorySpace.PSUM))
        p1 = ctx.enter_context(
            tc.tile_pool(name="p1", bufs=4, space=bass.MemorySpace.PSUM))

        # ---- constants / weights in SBUF -------------------------------
        ones8 = pc.tile([P, 1], FP8, tag="ones8")
        nc.vector.memset(ones8[:], 1.0)
        negone = pc.tile([P, 1], F32, tag="negone")
        nc.vector.memset(negone[:], -1.0)
        scratch = pc.tile([P, 512], BF16, tag="scratch")
        nc.gpsimd.memset(scratch[:], 0.0)

        # x first (LN1 is the critical path), per-token-tile chunks
        xt = px.tile([P, nt, C], F32, tag="x")
        x_r = x_d.rearrange("(t p) c -> t p c", p=P)
        for t in range(nt):
            nc.sync.dma_start(xt[:, t, :], x_r[t])

        # qkv weights per-k-chunk so the first matmuls can start early
        wqkvT = pwbig.tile([P, CT, OC], FP8, tag="wbig")
        wq_r = wqkvT_d.rearrange("(k p) o -> k p o", p=P)
        for ci in range(CT):
            nc.sync.dma_start(wqkvT[:, ci, :], wq_r[ci])

        a1s = pc.tile([P, HT], F32, tag="a1s")
        nc.sync.dma_start(a1s[:], a1s_d)
        b1s = pc.tile([P, HT], F32, tag="b1s")
        nc.sync.dma_start(b1s[:], b1s_d)
        cp1r = pc.tile([P, C], F32, tag="cp1r")
        nc.sync.dma_start(cp1r[:], cp1_d)
        c1r = pc.tile([P, C], F32, tag="c1r")
        nc.sync.dma_start(c1r[:], c1_d)
        wpT = pwp.tile([P, CT, C], FP8, tag="wp")
        nc.sync.dma_start(wpT[:], wpT_d.rearrange("(k p) o -> p k o", p=P))
        opt = {}
        for name, d in opt_d.items():
            opt[name] = pc.tile([P, C], F32, tag=name, name=f"t_{name}")
            nc.sync.dma_start(opt[name][:], d)

        # zero-padded v (per-head halves for pair-packed O accumulation)
        vza = pv.tile([P, nt, H // 2, P], FP8, tag="vza")
        vzb = pv.tile([P, nt, H // 2, P], FP8, tag="vzb")
        nc.gpsimd.memset(vza[:], 0.0)
        nc.gpsimd.memset(vzb[:], 0.0)

        # ---- stats tiles ------------------------------------------------
        musum = pstat.tile([P, nt], F32, tag="musum")
        nmu1 = pstat.tile([P, nt], F32, tag="nmu1")
        bn6 = pstat.tile([P, 2, 6], F32, tag="bn6")
        mv = pstat.tile([P, 2 * nt], F32, tag="mv")
        nmu2 = pstat.tile([P, nt], F32, tag="nmu2")
        vr2 = pstat.tile([P, nt], F32, tag="vr2")
        r2 = pstat.tile([P, nt], F32, tag="r2")
        rs2 = pstat.tile([P, nt], F32, tag="rs2")

        # HAM warm-up MMs; rhs depends on staggered LN1 tiles so they
        # spread through phase A instead of draining instantly.
        warm_n = [0]

        def warm(rhs_ap, k=1):
            for _ in range(k):
                wp_ = p1.tile([P, 512], F32, tag="b1",
                              name=f"warm{warm_n[0]}")
                warm_n[0] += 1
                nc.tensor.matmul(wp_[:], lhsT=scratch[:, 0:P], rhs=rhs_ap,
                                 start=True, stop=True)

        warm(scratch[:], 8)

        # ---- LN1 -> s1 (bf16) -> xbar transpose -> s1T fp8 -------------
        s1bf = ptok.tile([P, nt, C], BF16, tag="tok")
        s1T = pch.tile([P, CT, N], FP8, tag="ch")
        for t in range(nt):
            x_t = xt[:, t, :]
            nc.vector.tensor_reduce(musum[:, t:t + 1], x_t,
                                    axis=mybir.AxisListType.X, op=AL.add)
            nc.vector.tensor_scalar_mul(nmu1[:, t:t + 1], musum[:, t:t + 1],
                                        -1.0 / C)
            nc.scalar.activation(s1bf[:, t, :], x_t, AF.Sign,
                                 bias=nmu1[:, t:t + 1], scale=1.0)
            stg = ptb.tile([P, CT, P], BF16, tag="tb", name=f"stg1_{t}")
            nc.sync.dma_start_transpose(out=stg[:], in_=s1bf[:, t, :])
            nc.vector.tensor_copy(s1T[:, :, t * P:(t + 1) * P], stg[:])
            warm(s1bf[:, t, 0:512], 2)

        dbg_dump("s1T", s1T[:])

        # ---- qkv: q/k o-major (merged per-pair layout), v token-major ---
        # qkT[:, pr, :]  = [q_h0^T ; q_h1^T] (64+64 rows), +-1
        # kzT[:, pr, :]  = [k_h0^T ; k_h1^T], +-1
        qkT = pqk.tile([P, H // 2, N], FP8, tag="qk")
        kzT = pqk.tile([P, H // 2, N], FP8, tag="kz")

        for pr in range(H // 2):
            for qk, ot in ((0, pr), (1, 6 + pr)):
                ps = p2.tile([P, N], F32, tag="acc", name=f"zqk{ot}")
                for (n0, nsz) in _nchunks(N):
                    for j in range(CT // 2):
                        nc.tensor.matmul(
                            ps[:, n0:n0 + nsz],
                            lhsT=wqkvT[:, 2 * j:2 * j + 2, ot * P:(ot + 1) * P],
                            rhs=s1T[:, 2 * j:2 * j + 2, n0:n0 + nsz],
                            start=(j == 0), stop=(j == CT // 2 - 1),
                            perf_mode=DR)
                dst = qkT if qk == 0 else kzT
                nc.scalar.activation(dst[:, pr, :], ps[:], AF.Sign,
                                     bias=1.0, scale=2.0)

        # v: psum [tok, 768]; DVE +-1 evac in two strided ops per half
        for t in range(nt):
            ps = p2.tile([P, C], F32, tag="acc", name=f"zv{t}")
            for (o0, osz) in _nchunks(C):
                for j in range(CT // 2):
                    nc.tensor.matmul(
                        ps[:, o0:o0 + osz],
                        lhsT=s1T[:, 2 * j:2 * j + 2, t * P:(t + 1) * P],
                        rhs=wqkvT[:, 2 * j:2 * j + 2,
                                  2 * C + o0:2 * C + o0 + osz],
                        start=(j == 0), stop=(j == CT // 2 - 1), perf_mode=DR)
            ps_v = ps[:, 0:C].rearrange("p (h d) -> p h d", d=HD)
            # z even integer: (z > -1) == (z >= 0); {0,1} then 2x-1 -> +-1
            va = vza[:, t, :, 0:HD]
            vb = vzb[:, t, :, HD:P]
            nc.vector.tensor_scalar(va, ps_v[:, 0::2, :], -1.0, None,
                                    op0=AL.is_gt)
            nc.vector.tensor_scalar(va, va, 2.0, -1.0, op0=AL.mult, op1=AL.add)
            nc.vector.tensor_scalar(vb, ps_v[:, 1::2, :], -1.0, None,
                                    op0=AL.is_gt)
            nc.vector.tensor_scalar(vb, vb, 2.0, -1.0, op0=AL.mult, op1=AL.add)

        if dbg:
            dbg_dump("qkT", qkT[:])
            dbg_dump("kzT", kzT[:])
            dbg_dump("vza", vza[:])
            dbg_dump("vzb", vzb[:])

        # fc1 weights arrive during attention
        w1T = pwbig.tile([P, CT, HID], FP8, tag="wbig")
        nc.sync.dma_start(w1T[:], w1T_d.rearrange("(k p) o -> p k o", p=P))

        # ---- colsum of v per pair (bias for +-1-encoded pairs) ----------
        cb_all = pc.tile([P, H // 2], F32, tag="cball")
        for pr in range(H // 2):
            if pr in ACT_PAIRS:
                csp = p1.tile([P, 1], F32, tag="b1", name=f"csp{pr}")
                nmm = 0
                for mt in range(nt):
                    for vz in (vza, vzb):
                        nc.tensor.matmul(csp[:], lhsT=vz[:, mt, pr, :],
                                         rhs=ones8[:], start=(nmm == 0),
                                         stop=(nmm == 2 * nt - 1))
                        nmm += 1
                nc.scalar.activation(cb_all[:, pr:pr + 1], csp[:],
                                     AF.Identity, bias=1.0, scale=1.0)
            else:
                nc.vector.memset(cb_all[:, pr:pr + 1], 1.0)

        # ---- attention: 12 steps (nh, pair); S(i) overlaps O(i-1) -------
        soT = pch.tile([P, CT, N], FP8, tag="soT")
        steps = [(nh, pr) for nh in range(2) for pr in range(H // 2)]
        stq = {}
        ops = {}

        def emit_S_mt(i, mt):
            nh, pr = steps[i]
            st = stq[i]
            ps = p2.tile([P, N], F32, tag="acc", name=f"s{i}_{mt}")
            for hh in range(2):
                nc.tensor.matmul(
                    ps[:, hh * NH:(hh + 1) * NH],
                    lhsT=kzT[hh * HD:(hh + 1) * HD, pr, mt * P:(mt + 1) * P],
                    rhs=qkT[hh * HD:(hh + 1) * HD, pr,
                            nh * NH:(nh + 1) * NH],
                    start=True, stop=True)
            dst = st[:, mt, :, :]
            if pr in ACT_PAIRS:
                nc.scalar.activation(dst, ps[:], AF.Sign,
                                     bias=negone[:, 0:1], scale=1.0)
            else:
                nc.vector.tensor_scalar(dst, ps[:], 0.0, 2.0,
                                        op0=AL.is_gt, op1=AL.mult)

        def emit_O_j(i, j):
            nh, pr = steps[i]
            if j == 0:
                ops[i] = p1.tile([P, NH], F32, tag="b1", name=f"ot{i}")
            po = ops[i]
            st = stq[i]
            for hh, vz in ((0, vza), (1, vzb)):
                nc.tensor.matmul(
                    po[:],
                    lhsT=vz[:, 2 * j:2 * j + 2, pr, :],
                    rhs=st[:, 2 * j:2 * j + 2, hh, :],
                    start=(j == 0 and hh == 0),
                    stop=(j == nt // 2 - 1 and hh == 1), perf_mode=DR)

        def emit_O_tail(i):
            nh, pr = steps[i]
            po = ops.pop(i)
            stq.pop(i)
            nc.scalar.activation(soT[:, pr, nh * NH:(nh + 1) * NH], po[:],
                                 AF.Sign, bias=cb_all[:, pr:pr + 1], scale=1.0)

        for i in range(len(steps)):
            stq[i] = pst.tile([P, nt, 2, NH], FP8, tag="st", name=f"stq{i}")
            for mt in range(nt):
                emit_S_mt(i, mt)
                if i >= 1 and mt % 2 == 1:
                    emit_O_j(i - 1, mt // 2)
            if i >= 1:
                emit_O_tail(i - 1)
            if i == 1:
                # fc2 weights arrive during attention
                w2T = pwbig.tile([P, HT, C], FP8, tag="wbig")
                nc.sync.dma_start(w2T[:], w2T_d.rearrange("(k p) o -> p k o", p=P))
        i = len(steps) - 1
        for j in range(nt // 2):
            emit_O_j(i, j)
        emit_O_tail(i)
        dbg_dump("soT", soT[:])

        # ---- Y: proj + residual + LN2 per tile; fc1 n-halves; fc2 ------
        h2bf = ptok.tile([P, nt, C], BF16, tag="tok")
        h2T = pch.tile([P, CT, N], FP8, tag="ch")
        mgT = pmg.tile([P, HT, N], FP8, tag="mg")

        def emit_proj_ln2(t):
            x_t = xt[:, t, :]
            pa = p1.tile([P, 512], F32, tag="b1", name=f"prA{t}")
            pb = p1.tile([P, 256], F32, tag="b1", name=f"prB{t}")
            for ps_, o0, osz in ((pa, 0, 512), (pb, 512, 256)):
                for j in range(CT // 2):
                    nc.tensor.matmul(
                        ps_[:, 0:osz],
                        lhsT=soT[:, 2 * j:2 * j + 2, t * P:(t + 1) * P],
                        rhs=wpT[:, 2 * j:2 * j + 2, o0:o0 + osz],
                        start=(j == 0), stop=(j == CT // 2 - 1), perf_mode=DR)
            # x1 = x + psum * cp1 (+ cp2)
            nc.vector.tensor_tensor(pa[:], pa[:], cp1r[:, 0:512], op=AL.mult)
            nc.vector.tensor_tensor(pb[:], pb[:], cp1r[:, 512:768], op=AL.mult)
            nc.vector.tensor_tensor(x_t[:, 0:512], x_t[:, 0:512], pa[:],
                                    op=AL.add)
            nc.vector.tensor_tensor(x_t[:, 512:768], x_t[:, 512:768], pb[:],
                                    op=AL.add)
            if cfg.has_cp2:
                nc.vector.tensor_tensor(x_t, x_t, opt["cp2r"][:], op=AL.add)
            # LN2 stats
            nc.vector.bn_stats(bn6[:, 0, :], x_t[:, :C // 2])
            nc.vector.bn_stats(bn6[:, 1, :], x_t[:, C // 2:])
            nc.vector.bn_aggr(mv[:, 2 * t:2 * t + 2], bn6[:])
            nc.vector.tensor_scalar_add(vr2[:, t:t + 1],
                                        mv[:, 2 * t + 1:2 * t + 2], 1e-5)
            nc.vector.reciprocal(r2[:, t:t + 1], vr2[:, t:t + 1])
            nc.scalar.activation(rs2[:, t:t + 1], r2[:, t:t + 1], AF.Sqrt,
                                 bias=0.0, scale=1.0)
            nc.vector.tensor_scalar_mul(nmu2[:, t:t + 1],
                                        mv[:, 2 * t:2 * t + 1], -1.0)
            nc.vector.tensor_scalar(h2bf[:, t, :], x_t, nmu2[:, t:t + 1],
                                    rs2[:, t:t + 1], op0=AL.add, op1=AL.mult)
            stg = ptb.tile([P, CT, P], BF16, tag="tb", name=f"stg2_{t}")
            nc.sync.dma_start_transpose(out=stg[:], in_=h2bf[:, t, :])
            nc.vector.tensor_copy(h2T[:, :, t * P:(t + 1) * P], stg[:])
            warm(h2bf[:, t, 0:512], 1)

        def emit_fc1(nh):
            for ht in range(HT):
                ps = p2.tile([P, NH], F32, tag="acc", name=f"f1_{nh}_{ht}")
                for j in range(CT // 2):
                    nc.tensor.matmul(
                        ps[:],
                        lhsT=w1T[:, 2 * j:2 * j + 2, ht * P:(ht + 1) * P],
                        rhs=h2T[:, 2 * j:2 * j + 2, nh * NH:(nh + 1) * NH],
                        start=(j == 0), stop=(j == CT // 2 - 1), perf_mode=DR)
                nc.scalar.activation(mgT[:, ht, nh * NH:(nh + 1) * NH], ps[:],
                                     GELU_FN, bias=b1s[:, ht:ht + 1],
                                     scale=a1s[:, ht:ht + 1])

        def emit_fc2(t):
            x_t = xt[:, t, :]
            pa = p1.tile([P, 512], F32, tag="b1", name=f"f2A{t}")
            pb = p1.tile([P, 256], F32, tag="b1", name=f"f2B{t}")
            for ps_, o0, osz in ((pa, 0, 512), (pb, 512, 256)):
                for j in range(HT // 2):
                    nc.tensor.matmul(
                        ps_[:, 0:osz],
                        lhsT=mgT[:, 2 * j:2 * j + 2, t * P:(t + 1) * P],
                        rhs=w2T[:, 2 * j:2 * j + 2, o0:o0 + osz],
                        start=(j == 0), stop=(j == HT // 2 - 1), perf_mode=DR)
            nc.vector.tensor_tensor(pa[:], pa[:], c1r[:, 0:512], op=AL.mult)
            nc.vector.tensor_tensor(pb[:], pb[:], c1r[:, 512:768], op=AL.mult)
            nc.vector.tensor_tensor(x_t[:, 0:512], x_t[:, 0:512], pa[:],
                                    op=AL.add)
            nc.vector.tensor_tensor(x_t[:, 512:768], x_t[:, 512:768], pb[:],
                                    op=AL.add)
            if cfg.has_c2:
                nc.vector.tensor_tensor(x_t, x_t, opt["c2r"][:], op=AL.add)
            nc.sync.dma_start(
                out_d.rearrange("(t p) c -> t p c", p=P)[t], x_t)

        for t in range(4):
            emit_proj_ln2(t)
        emit_fc1(0)
        for t in range(4, nt):
            emit_proj_ln2(t)
        emit_fc1(1)
        dbg_dump("h2T", h2T[:])
        for t in range(nt):
            emit_fc2(t)

    nc.compile()
    input_names = ["x", "wqkvT", "wpT", "w1T", "w2T", "a1s", "b1s",
                   "cp1r", "c1r"] + list(opt_d.keys())
    if dbg:
        return nc, input_names, dbg_t
    return nc, input_names


# -------------------------------------------------------------------------
# host-side prep + execution
# -------------------------------------------------------------------------

def _sgn(a):
    return np.where(a >= 0, np.float32(1.0), np.float32(-1.0))


def prep_host_inputs(inputs, cfg: Cfg):
    """Returns dict of per-core-common host arrays keyed by dram names."""
    f8 = ml_dtypes.float8_e4m3
    w_qkv = np.asarray(inputs["w_qkv"], np.float32)
    w_proj = np.asarray(inputs["w_proj"], np.float32)
    w_fc1 = np.asarray(inputs["w_fc1"], np.float32)
    w_fc2 = np.asarray(inputs["w_fc2"], np.float32)
    ls1 = np.asarray(inputs["ls1_g"], np.float32)
    ls2 = np.asarray(inputs["ls2_g"], np.float32)
    b_proj = np.asarray(inputs["b_proj"], np.float32)
    b_fc1 = np.asarray(inputs["b_fc1"], np.float32)
    b_fc2 = np.asarray(inputs["b_fc2"], np.float32)

    ap = np.abs(w_proj).mean(axis=1)    # [C] alpha_proj
    a1 = np.abs(w_fc1).mean(axis=1)     # [HID]
    a2 = np.abs(w_fc2).mean(axis=1)     # [C]

    d = {
        "wqkvT": np.ascontiguousarray(_sgn(w_qkv).T).astype(f8),
        "wpT": np.ascontiguousarray(_sgn(w_proj).T).astype(f8),
        "w1T": np.ascontiguousarray(_sgn(w_fc1).T).astype(f8),
        "w2T": np.ascontiguousarray(_sgn(w_fc2).T).astype(f8),
        "a1s": np.ascontiguousarray(a1.reshape(HT, P).T),
        "b1s": np.ascontiguousarray(b_fc1.reshape(HT, P).T),
        # wpT/w2T carry only signs (fp8); per-out-channel scales applied on
        # device: proj via cp1r = ls1*alpha_p, fc2 via c1r = ls2*alpha2.
        "cp1r": np.ascontiguousarray(
            np.broadcast_to(ls1 * ap, (P, C)).copy()),
        "c1r": np.ascontiguousarray(
            np.broadcast_to(ls2 * a2, (P, C)).copy()),
    }
    if cfg.has_cp2:
        d["cp2r"] = np.ascontiguousarray(np.broadcast_to(ls1 * b_proj, (P, C)).copy())
    if cfg.has_c2:
        d["c2r"] = np.ascontiguousarray(np.broadcast_to(ls2 * b_fc2, (P, C)).copy())
    if not cfg.ln1_fast:
        d["g1r"] = np.ascontiguousarray(
            np.broadcast_to(np.asarray(inputs["ln1_g"], np.float32), (P, C)).copy())
        d["b1r"] = np.ascontiguousarray(
            np.broadcast_to(np.asarray(inputs["ln1_b"], np.float32), (P, C)).copy())
    if not cfg.ln2_fast:
        d["g2r"] = np.ascontiguousarray(
            np.broadcast_to(np.asarray(inputs["ln2_g"], np.float32), (P, C)).copy())
        d["b2r"] = np.ascontiguousarray(
            np.broadcast_to(np.asarray(inputs["ln2_b"], np.float32), (P, C)).copy())
    return d


def make_cfg(inputs, nt=8):
    ln1_g = np.asarray(inputs["ln1_g"], np.float32)
    ln1_b = np.asarray(inputs["ln1_b"], np.float32)
    ln2_g = np.asarray(inputs["ln2_g"], np.float32)
    ln2_b = np.asarray(inputs["ln2_b"], np.float32)
    ls1 = np.asarray(inputs["ls1_g"], np.float32)
    ls2 = np.asarray(inputs["ls2_g"], np.float32)
    b_proj = np.asarray(inputs["b_proj"], np.float32)
    b_fc2 = np.asarray(inputs["b_fc2"], np.float32)
    return Cfg(
        nt=nt,
        ln1_fast=bool(np.all(ln1_b == 0) and np.all(ln1_g > 0)),
        ln2_fast=bool(np.all(ln2_g == 1) and np.all(ln2_b == 0)),
        has_cp2=bool(np.any(ls1 * b_proj != 0)),
        has_c2=bool(np.any(ls2 * b_fc2 != 0)),
    )


_PROG_CACHE = {}


def get_program(cfg: Cfg):
    key = cfg
    if key not in _PROG_CACHE:
        _PROG_CACHE[key] = build_program_v2(cfg)
    return _PROG_CACHE[key]


def kernel(**inputs):
    from concourse.bass_utils import run_bass_kernel_spmd

    x = np.asarray(inputs["x"], np.float32)
    assert x.shape == (B, 1024, C), x.shape
    cfg = make_cfg(inputs, nt=1024 // P)
    nc, _names = get_program(cfg)
    common = prep_host_inputs(inputs, cfg)

    in_maps = []
    for b in range(B):
        m = dict(common)
        m["x"] = np.ascontiguousarray(x[b])
        in_maps.append(m)

    res = run_bass_kernel_spmd(nc, in_maps, core_ids=list(range(N_CORES)))
    out = np.stack([res.results[b]["out"] for b in range(B)], axis=0)
    return out.astype(np.float32)
